# revision 1
# baseline (speedup 1.0000x reference)
"""Trainium2 Bass kernel for KMeans assignment (argmin over 8192 centroids).

Problem: x [32768, 1024] f32, centroids [1024, 8192] f32 ->
         argmin_k ||x_n - c_k||^2  as int32 [32768].

Math: argmin_k (||x||^2 - 2 x.c_k + ||c_k||^2) == argmax_k (x.c_k - 0.5*||c_k||^2).
The ||x||^2 term is row-constant and drops out of the argmin.

Sharding: data-parallel over N across 8 cores (4096 rows each), centroids
replicated. Per core: scores = xT_shard.T @ centroids + bias computed in
16 chunks of 512 centroids; per chunk the DVE max8/max_index ops produce
the chunk top-8 values + indices per row. The 16 chunk winners per row are
merged on the host (trivial numpy argmax over 16 candidates).

Matmul runs in float32r (TF32-like ~11 mantissa bits, 1 cycle/row vs 4 for
fp32). Optional exact refinement: the host re-scores each row's top
candidates in fp32/fp64 to undo f32r rounding on near-ties.
"""
import os
import numpy as np

# ---- problem constants (hardcoded per harness contract) ----
N_FULL, D, K = 32768, 1024, 8192
N_CORES = 8
NC = N_FULL // N_CORES          # 4096 rows per core
NB = 2                          # n-blocks per core
NBLK = NC // NB                 # 2048 rows per block
NT = NBLK // 128                # 16 row-tiles per block
CHUNK = int(os.environ.get("KMEANS_CHUNK", "512"))  # centroid chunk
KC = K // CHUNK
DC = D // 128                   # 8 contraction chunks

_compiled = {}


def _build(mode: str):
    """Build + compile the per-core Bass program. Returns the Bass object."""
    from contextlib import ExitStack
    import concourse.bacc as bacc
    import concourse.mybir as mybir
    import concourse.tile as tile

    f32 = mybir.dt.float32
    f32r = mybir.dt.float32r
    bf16 = mybir.dt.bfloat16
    u32 = mybir.dt.uint32

    nc = bacc.Bacc("TRN2", target_bir_lowering=False, debug=False)

    if mode == "f32r":
        mm_dt = f32r
        xt_d = [nc.dram_tensor("xt", [D, NC], f32r, kind="ExternalInput").ap()]
        c_d = [nc.dram_tensor("cent", [D, K], f32r, kind="ExternalInput").ap()]
        NMAT = [(0, 0)]
    else:  # bf16x2: hi/lo split, 3 matmuls
        mm_dt = bf16
        xt_d = [nc.dram_tensor(f"xt{i}", [D, NC], bf16, kind="ExternalInput").ap()
                for i in range(2)]
        c_d = [nc.dram_tensor(f"cent{i}", [D, K], bf16, kind="ExternalInput").ap()
               for i in range(2)]
        NMAT = [(0, 0), (0, 1), (1, 0)]
    nin = len(xt_d)

    bias_d = nc.dram_tensor("bias", [128, K], f32, kind="ExternalInput").ap()
    outv_d = nc.dram_tensor("outv", [NB, 128, NT * KC * 8], f32,
                            kind="ExternalOutput").ap()
    outi_d = nc.dram_tensor("outi", [NB, 128, NT * KC * 8], u32,
                            kind="ExternalOutput").ap()

    with tile.TileContext(nc) as tc:
        with ExitStack() as ctx:
            const_pool = ctx.enter_context(tc.tile_pool(name="const", bufs=1))
            xt_pool = ctx.enter_context(tc.tile_pool(name="xt", bufs=1))
            c_pool = ctx.enter_context(tc.tile_pool(name="cent", bufs=2))
            sc_pool = ctx.enter_context(tc.tile_pool(name="scores", bufs=4))
            acc_pool = ctx.enter_context(tc.tile_pool(name="acc", bufs=2))
            ps_pool = ctx.enter_context(tc.tile_pool(name="psum", bufs=(4 if CHUNK == 512 else 3), space="PSUM"))

            bias_sb = const_pool.tile([128, K], f32, name="bias_sb")
            nc.sync.dma_start(bias_sb[:], bias_d[:])

            for b in range(NB):
                # load x^T block: DC chunks of [128, NBLK] per input part
                xt_sb = [xt_pool.tile([128, DC * NBLK], mm_dt, name=f"xt_sb{i}",
                                      tag=f"xt{i}") for i in range(nin)]
                for i in range(nin):
                    for d in range(DC):
                        nc.sync.dma_start(
                            xt_sb[i][:, d * NBLK:(d + 1) * NBLK],
                            xt_d[i][d * 128:(d + 1) * 128, b * NBLK:(b + 1) * NBLK])

                mv_all = acc_pool.tile([128, NT * KC * 8], f32, name="mv_all", tag="mv")
                mi_all = acc_pool.tile([128, NT * KC * 8], u32, name="mi_all", tag="mi")

                for kc in range(KC):
                    c_sb = [c_pool.tile([128, DC * CHUNK], mm_dt, name=f"c_sb{i}",
                                        tag=f"c{i}") for i in range(nin)]
                    for i in range(nin):
                        for d in range(DC):
                            nc.sync.dma_start(
                                c_sb[i][:, d * CHUNK:(d + 1) * CHUNK],
                                c_d[i][d * 128:(d + 1) * 128,
                                       kc * CHUNK:(kc + 1) * CHUNK])
                    NSUB = CHUNK // 512
                    for nt in range(NT):
                        ps = ps_pool.tile([128, CHUNK], f32, name="ps")
                        nmm = len(NMAT) * DC
                        for d in range(DC):
                            for (ix, ic) in NMAT:
                                for j in range(NSUB):
                                    nc.tensor.matmul(
                                        ps[:, j * 512:(j + 1) * 512],
                                        xt_sb[ix][:, d * NBLK + nt * 128:
                                                  d * NBLK + (nt + 1) * 128],
                                        c_sb[ic][:, d * CHUNK + j * 512:
                                                 d * CHUNK + (j + 1) * 512],
                                        start=(d == 0 and (ix, ic) == NMAT[0]),
                                        stop=(d == DC - 1 and (ix, ic) == NMAT[-1]))
                        sc = sc_pool.tile([128, CHUNK], f32, name="sc")
                        nc.vector.tensor_tensor(
                            sc[:], ps[:], bias_sb[:, kc * CHUNK:(kc + 1) * CHUNK],
                            mybir.AluOpType.add)
                        col = nt * KC * 8 + kc * 8
                        nc.vector.max(mv_all[:, col:col + 8], sc[:])
                        nc.vector.max_index(mi_all[:, col:col + 8],
                                            mv_all[:, col:col + 8], sc[:])

                nc.sync.dma_start(outv_d[b], mv_all[:])
                nc.sync.dma_start(outi_d[b], mi_all[:])
    nc.compile()
    return nc


def _get_nc(mode: str):
    if mode not in _compiled:
        _compiled[mode] = _build(mode)
    return _compiled[mode]


def _merge_host(outv, outi):
    """Merge per-chunk top-1 candidates -> global argmax indices [NC]."""
    # outv/outi: [NB, 128, NT*KC*8]
    vals = outv.reshape(NB, 128, NT, KC, 8).transpose(0, 2, 1, 3, 4)
    idxs = outi.reshape(NB, 128, NT, KC, 8).transpose(0, 2, 1, 3, 4)
    vals = vals.reshape(NC, KC, 8)
    idxs = idxs.reshape(NC, KC, 8)
    v0 = vals[:, :, 0]
    i0 = idxs[:, :, 0].astype(np.int64)
    am = np.argmax(v0, axis=1)            # first occurrence on ties
    rows = np.arange(NC)
    gi = am * CHUNK + i0[rows, am]
    return gi.astype(np.int32), vals, idxs


def kernel(x: np.ndarray, centroids: np.ndarray) -> np.ndarray:
    mode = os.environ.get("KMEANS_MM_MODE", "f32r")
    refine = int(os.environ.get("KMEANS_REFINE", "1"))
    from concourse.bass_utils import run_bass_kernel_spmd

    x = np.asarray(x, dtype=np.float32)
    centroids = np.asarray(centroids, dtype=np.float32)
    nc = _get_nc(mode)

    xt = np.ascontiguousarray(x.T)                       # [D, N]
    bias_row = -0.5 * np.einsum("dk,dk->k", centroids, centroids,
                                dtype=np.float64).astype(np.float32)
    bias = np.ascontiguousarray(np.broadcast_to(bias_row, (128, K)))

    in_maps = []
    for c in range(N_CORES):
        sl = np.ascontiguousarray(xt[:, c * NC:(c + 1) * NC])
        if mode == "f32r":
            m = {"xt": sl, "cent": centroids, "bias": bias}
        else:
            import ml_dtypes
            xh = sl.astype(ml_dtypes.bfloat16)
            xl = (sl - xh.astype(np.float32)).astype(ml_dtypes.bfloat16)
            ch = centroids.astype(ml_dtypes.bfloat16)
            cl = (centroids - ch.astype(np.float32)).astype(ml_dtypes.bfloat16)
            m = {"xt0": xh, "xt1": xl, "cent0": ch, "cent1": cl, "bias": bias}
        in_maps.append(m)

    res = run_bass_kernel_spmd(nc, in_maps, core_ids=list(range(N_CORES)))

    out = np.empty(N_FULL, dtype=np.int32)
    for c in range(N_CORES):
        gi, vals, idxs = _merge_host(res.results[c]["outv"], res.results[c]["outi"])
        if refine:
            gi = _refine(x[c * NC:(c + 1) * NC], centroids, bias_row, vals, idxs)
        out[c * NC:(c + 1) * NC] = gi
    return out


def _refine(xs, centroids, bias_row, vals, idxs, top=8):
    """Re-score each row's top candidates exactly in fp32 to undo f32r rounding."""
    n = xs.shape[0]
    fv = vals.reshape(n, KC * 8)
    fi = (idxs.astype(np.int64)
          + (np.arange(KC) * CHUNK)[None, :, None]).reshape(n, KC * 8)
    part = np.argpartition(-fv, top - 1, axis=1)[:, :top]
    cand = np.take_along_axis(fi, part, axis=1)          # [n, top] global idx
    # exact scores for candidates, batched
    out = np.empty(n, dtype=np.int32)
    bs = 4096
    for s in range(0, n, bs):
        e = min(s + bs, n)
        cb = cand[s:e]                                   # [b, top]
        cc = centroids.T[cb]                             # [b, top, D]
        sc = np.einsum("bd,btd->bt", xs[s:e], cc, dtype=np.float64)
        sc = sc + bias_row[cb]
        # argmax with ties -> smallest global index (first occurrence in k)
        best = sc.max(axis=1, keepdims=True)
        big = np.where(sc >= best, cb, np.iinfo(np.int64).max)
        out[s:e] = big.min(axis=1).astype(np.int32)
    return out



# revision 2
# speedup vs baseline: 1.0510x; 1.0510x over previous
"""Trainium2 Bass kernel for KMeans assignment (argmin over 8192 centroids).

Problem: x [32768, 1024] f32, centroids [1024, 8192] f32 ->
         argmin_k ||x_n - c_k||^2  as int32 [32768].

Math: argmin_k (||x||^2 - 2 x.c_k + ||c_k||^2) == argmax_k (x.c_k - 0.5*||c_k||^2).

v4 (fixes v3's HAM collapse): fp8-e4m3 DoubleRow matmuls with NB=1 --
all of x^T resident in SBUF (fp8 halves it to 32KB/partition), so there
is no mid-kernel block boundary: v3's two ~4us PE gaps at the b0->b1
transition dropped the PE clock gate to K=4/8 for the remaining 692us
and it never re-warmed. With a single block the PE stream never pauses.

Also: PSUM banks are paired -- each psum tile is [128, 1024] (2 banks),
filled by two 512-col accumulation groups (4 DoubleRow MMs + 1 f32r
bias MM each); DVE then runs one max8 + one max_index over the full
1024 columns, halving per-instruction overhead and the DVE's PSUM-read
count. Host merges per-1024-chunk top-8 candidates, exact-rescoring the
global top-16 in fp64 (verified 0 mismatches in simulation).
"""
import numpy as np

# ---- problem constants (hardcoded per harness contract) ----
N_FULL, D, K = 32768, 1024, 8192
N_CORES = 8
NC = N_FULL // N_CORES          # 4096 rows per core
NT = NC // 128                  # 32 row-tiles
CHUNK = 512                     # one matmul / PSUM bank
PAIR = 2 * CHUNK                # 1024: one psum tile, max8 span
KCM = K // PAIR                 # 8 chunk-pairs
DC = D // 128                   # 8 contraction chunks
DP = DC // 2                    # 4 DoubleRow pairs

_compiled = {}


def _build():
    """Build + compile the per-core Bass program. Returns the Bass object."""
    from contextlib import ExitStack
    import concourse.bacc as bacc
    import concourse.mybir as mybir
    import concourse.tile as tile

    f32 = mybir.dt.float32
    f32r = mybir.dt.float32r
    fp8 = mybir.dt.float8e4
    u32 = mybir.dt.uint32
    DR = mybir.MatmulPerfMode.DoubleRow

    nc = bacc.Bacc("TRN2", target_bir_lowering=False, debug=False)

    xt_d = nc.dram_tensor("xt", [D, NC], fp8, kind="ExternalInput").ap()
    c_d = nc.dram_tensor("cent", [D, K], fp8, kind="ExternalInput").ap()
    bias_d = nc.dram_tensor("bias", [1, K], f32r, kind="ExternalInput").ap()
    ones_d = nc.dram_tensor("ones", [1, 128], f32r, kind="ExternalInput").ap()
    outv_d = nc.dram_tensor("outv", [KCM, 128, NT * 8], f32,
                            kind="ExternalOutput").ap()
    outi_d = nc.dram_tensor("outi", [KCM, 128, NT * 8], u32,
                            kind="ExternalOutput").ap()

    with tile.TileContext(nc) as tc:
        with ExitStack() as ctx:
            const_pool = ctx.enter_context(tc.tile_pool(name="const", bufs=1))
            xt_pool = ctx.enter_context(tc.tile_pool(name="xt", bufs=1))
            c_pool = ctx.enter_context(tc.tile_pool(name="cent", bufs=2))
            acc_pool = ctx.enter_context(tc.tile_pool(name="acc", bufs=3))
            ps_pool = ctx.enter_context(
                tc.tile_pool(name="psum", bufs=3, space="PSUM"))

            bias_sb = const_pool.tile([1, K], f32r, name="bias_sb")
            nc.sync.dma_start(bias_sb[:], bias_d[:])
            ones_sb = const_pool.tile([1, 128], f32r, name="ones_sb")
            nc.sync.dma_start(ones_sb[:], ones_d[:])

            # full x^T resident: [128, DC, NC] fp8 = 32KB/partition
            xt_sb = xt_pool.tile([128, DC, NC], fp8, name="xt_sb")
            for d in range(DC):
                nc.sync.dma_start(xt_sb[:, d, :],
                                  xt_d[d * 128:(d + 1) * 128, :])

            for kcp in range(KCM):
                c_sb = c_pool.tile([128, DC, PAIR], fp8, name="c_sb", tag="c")
                for d in range(DC):
                    nc.sync.dma_start(
                        c_sb[:, d, :],
                        c_d[d * 128:(d + 1) * 128,
                            kcp * PAIR:(kcp + 1) * PAIR])

                mv = acc_pool.tile([128, NT * 8], f32, name="mv", tag="mv")
                mi = acc_pool.tile([128, NT * 8], u32, name="mi", tag="mi")

                for nt in range(NT):
                    ps = ps_pool.tile([128, PAIR], f32, name="ps")
                    # j outer / half inner: consecutive matmuls share the
                    # same stationary weights (one LDWEIGHTS per 2 streams)
                    for j in range(DP):
                        for half in range(2):
                            hs = half * CHUNK
                            nc.tensor.matmul(
                                ps[:, hs:hs + CHUNK],
                                xt_sb[:, 2 * j:2 * j + 2,
                                      nt * 128:(nt + 1) * 128],
                                c_sb[:, 2 * j:2 * j + 2, hs:hs + CHUNK],
                                start=(j == 0),
                                stop=False,
                                perf_mode=DR)
                    for half in range(2):
                        hs = half * CHUNK
                        # bias add on PE: ones[1,128].T @ bias[1,512]
                        nc.tensor.matmul(
                            ps[:, hs:hs + CHUNK],
                            ones_sb[:],
                            bias_sb[:, kcp * PAIR + hs:
                                    kcp * PAIR + hs + CHUNK],
                            start=False,
                            stop=True)
                    col = nt * 8
                    nc.vector.max(mv[:, col:col + 8], ps[:])
                    nc.vector.max_index(mi[:, col:col + 8],
                                        mv[:, col:col + 8], ps[:])

                nc.sync.dma_start(outv_d[kcp], mv[:])
                nc.sync.dma_start(outi_d[kcp], mi[:])
    nc.compile()
    return nc


def _get_nc(mode: str = "v4"):
    if mode not in _compiled:
        _compiled[mode] = _build()
    return _compiled[mode]


def _make_in_maps(x, centroids):
    import ml_dtypes
    xt = np.ascontiguousarray(x.T)                       # [D, N]
    bias_row = -0.5 * np.einsum("dk,dk->k", centroids, centroids,
                                dtype=np.float64).astype(np.float32)
    bias = np.ascontiguousarray(bias_row.reshape(1, K))
    ones = np.ones((1, 128), dtype=np.float32)
    cq = centroids.astype(ml_dtypes.float8_e4m3fn)
    in_maps = []
    for c in range(N_CORES):
        sl = np.ascontiguousarray(xt[:, c * NC:(c + 1) * NC]).astype(
            ml_dtypes.float8_e4m3fn)
        in_maps.append({"xt": sl, "cent": cq, "bias": bias, "ones": ones})
    return in_maps, bias_row


def _merge_host(outv, outi):
    """[KCM, 128, NT*8] device layout -> [NC, KCM, 8] candidates."""
    vals = outv.reshape(KCM, 128, NT, 8).transpose(2, 1, 0, 3).reshape(
        NC, KCM, 8)
    idxs = outi.reshape(KCM, 128, NT, 8).transpose(2, 1, 0, 3).reshape(
        NC, KCM, 8)
    return vals, idxs


def kernel(x: np.ndarray, centroids: np.ndarray) -> np.ndarray:
    from concourse.bass_utils import run_bass_kernel_spmd

    x = np.asarray(x, dtype=np.float32)
    centroids = np.asarray(centroids, dtype=np.float32)
    nc = _get_nc()

    in_maps, bias_row = _make_in_maps(x, centroids)
    res = run_bass_kernel_spmd(nc, in_maps, core_ids=list(range(N_CORES)))

    out = np.empty(N_FULL, dtype=np.int32)
    for c in range(N_CORES):
        vals, idxs = _merge_host(res.results[c]["outv"],
                                 res.results[c]["outi"])
        gi = _refine(x[c * NC:(c + 1) * NC], centroids, bias_row, vals, idxs)
        out[c * NC:(c + 1) * NC] = gi
    return out


def _refine(xs, centroids, bias_row, vals, idxs, top=16):
    """Re-score each row's top candidates exactly to undo fp8 noise."""
    n = xs.shape[0]
    fv = vals.reshape(n, KCM * 8)
    fi = (idxs.astype(np.int64)
          + (np.arange(KCM) * PAIR)[None, :, None]).reshape(n, KCM * 8)
    part = np.argpartition(-fv, top - 1, axis=1)[:, :top]
    cand = np.take_along_axis(fi, part, axis=1)          # [n, top] global idx
    out = np.empty(n, dtype=np.int32)
    bs = 4096
    for s in range(0, n, bs):
        e = min(s + bs, n)
        cb = cand[s:e]                                   # [b, top]
        cc = centroids.T[cb]                             # [b, top, D]
        sc = np.einsum("bd,btd->bt", xs[s:e], cc, dtype=np.float64)
        sc = sc + bias_row[cb]
        # argmax with ties -> smallest global index (first occurrence in k)
        best = sc.max(axis=1, keepdims=True)
        big = np.where(sc >= best, cb, np.iinfo(np.int64).max)
        out[s:e] = big.min(axis=1).astype(np.int32)
    return out


# revision 3
# speedup vs baseline: 1.0741x; 1.0220x over previous
"""Trainium2 Bass kernel for KMeans assignment (argmin over 8192 centroids).

Problem: x [32768, 1024] f32, centroids [1024, 8192] f32 ->
         argmin_k ||x_n - c_k||^2  as int32 [32768].

Math: argmin_k (||x||^2 - 2 x.c_k + ||c_k||^2) == argmax_k (x.c_k - 0.5*||c_k||^2).

v4 (fixes v3's HAM collapse): fp8-e4m3 DoubleRow matmuls with NB=1 --
all of x^T resident in SBUF (fp8 halves it to 32KB/partition), so there
is no mid-kernel block boundary: v3's two ~4us PE gaps at the b0->b1
transition dropped the PE clock gate to K=4/8 for the remaining 692us
and it never re-warmed. With a single block the PE stream never pauses.

Also: PSUM banks are paired -- each psum tile is [128, 1024] (2 banks),
filled by two 512-col accumulation groups (4 DoubleRow MMs + 1 f32r
bias MM each); DVE then runs one max8 + one max_index over the full
1024 columns, halving per-instruction overhead and the DVE's PSUM-read
count. Host merges per-1024-chunk top-8 candidates, exact-rescoring the
global top-16 in fp64 (verified 0 mismatches in simulation).
"""
import numpy as np

# ---- problem constants (hardcoded per harness contract) ----
N_FULL, D, K = 32768, 1024, 8192
N_CORES = 8
NC = N_FULL // N_CORES          # 4096 rows per core
NT = NC // 128                  # 32 row-tiles
CHUNK = 512                     # one matmul / PSUM bank
PAIR = 2 * CHUNK                # 1024: one psum tile, max8 span
KCM = K // PAIR                 # 8 chunk-pairs
DC = D // 128                   # 8 contraction chunks
DP = DC // 2                    # 4 DoubleRow pairs

_compiled = {}


def _build():
    """Build + compile the per-core Bass program. Returns the Bass object."""
    from contextlib import ExitStack
    import concourse.bacc as bacc
    import concourse.mybir as mybir
    import concourse.tile as tile

    f32 = mybir.dt.float32
    f32r = mybir.dt.float32r
    fp8 = mybir.dt.float8e4
    u32 = mybir.dt.uint32
    DR = mybir.MatmulPerfMode.DoubleRow

    nc = bacc.Bacc("TRN2", target_bir_lowering=False, debug=False)

    xt_d = nc.dram_tensor("xt", [D, NC], fp8, kind="ExternalInput").ap()
    c_d = nc.dram_tensor("cent", [D, K], fp8, kind="ExternalInput").ap()
    bias2_d = nc.dram_tensor("bias2", [1, 2, K], fp8,
                             kind="ExternalInput").ap()
    onesq_d = nc.dram_tensor("onesq", [1, 2, 128], fp8,
                             kind="ExternalInput").ap()
    outv_d = nc.dram_tensor("outv", [KCM, 128, NT * 8], f32,
                            kind="ExternalOutput").ap()
    outi_d = nc.dram_tensor("outi", [KCM, 128, NT * 8], u32,
                            kind="ExternalOutput").ap()

    with tile.TileContext(nc) as tc:
        with ExitStack() as ctx:
            const_pool = ctx.enter_context(tc.tile_pool(name="const", bufs=1))
            xt_pool = ctx.enter_context(tc.tile_pool(name="xt", bufs=1))
            c_pool = ctx.enter_context(tc.tile_pool(name="cent", bufs=2))
            acc_pool = ctx.enter_context(tc.tile_pool(name="acc", bufs=3))
            ps_pool = ctx.enter_context(
                tc.tile_pool(name="psum", bufs=3, space="PSUM"))

            bias2_sb = const_pool.tile([1, 2, K], fp8, name="bias2_sb")
            nc.sync.dma_start(bias2_sb[:], bias2_d[:])
            onesq_sb = const_pool.tile([1, 2, 128], fp8, name="onesq_sb")
            nc.sync.dma_start(onesq_sb[:], onesq_d[:])

            # full x^T resident: [128, DC, NC] fp8 = 32KB/partition
            xt_sb = xt_pool.tile([128, DC, NC], fp8, name="xt_sb")
            for d in range(DC):
                nc.sync.dma_start(xt_sb[:, d, :],
                                  xt_d[d * 128:(d + 1) * 128, :])

            for kcp in range(KCM):
                c_sb = c_pool.tile([128, DC, PAIR], fp8, name="c_sb", tag="c")
                for d in range(DC):
                    nc.sync.dma_start(
                        c_sb[:, d, :],
                        c_d[d * 128:(d + 1) * 128,
                            kcp * PAIR:(kcp + 1) * PAIR])

                mv = acc_pool.tile([128, NT * 8], f32, name="mv", tag="mv")
                mi = acc_pool.tile([128, NT * 8], u32, name="mi", tag="mi")

                for nt in range(NT):
                    ps = ps_pool.tile([128, PAIR], f32, name="ps")
                    # j outer / half inner: consecutive matmuls share the
                    # same stationary weights (one LDWEIGHTS per 2 streams)
                    for j in range(DP):
                        for half in range(2):
                            hs = half * CHUNK
                            nc.tensor.matmul(
                                ps[:, hs:hs + CHUNK],
                                xt_sb[:, 2 * j:2 * j + 2,
                                      nt * 128:(nt + 1) * 128],
                                c_sb[:, 2 * j:2 * j + 2, hs:hs + CHUNK],
                                start=(j == 0),
                                stop=False,
                                perf_mode=DR)
                    for half in range(2):
                        hs = half * CHUNK
                        # bias on PE via DoubleRow: (1/s)*(bias_hi+bias_lo),
                        # mean-centered (uniform shift leaves argmax alone)
                        nc.tensor.matmul(
                            ps[:, hs:hs + CHUNK],
                            onesq_sb[:],
                            bias2_sb[:, :, kcp * PAIR + hs:
                                     kcp * PAIR + hs + CHUNK],
                            start=False,
                            stop=True,
                            perf_mode=DR)
                    col = nt * 8
                    nc.vector.max(mv[:, col:col + 8], ps[:])
                    nc.vector.max_index(mi[:, col:col + 8],
                                        mv[:, col:col + 8], ps[:])

                nc.sync.dma_start(outv_d[kcp], mv[:])
                nc.sync.dma_start(outi_d[kcp], mi[:])
    nc.compile()
    return nc


def _get_nc(mode: str = "v4"):
    if mode not in _compiled:
        _compiled[mode] = _build()
    return _compiled[mode]


def _make_in_maps(x, centroids):
    import ml_dtypes
    fp8 = ml_dtypes.float8_e4m3fn
    xt = np.ascontiguousarray(x.T)                       # [D, N]
    bias_row = -0.5 * np.einsum("dk,dk->k", centroids, centroids,
                                dtype=np.float64).astype(np.float32)
    # mean-center (uniform shift leaves argmax unchanged), split into
    # two e4m3 terms at a power-of-2 scale
    bc = (bias_row - bias_row.mean()).astype(np.float64)
    s = 4.0
    while s > 1.0 and np.abs(bc * s).max() > 400.0:
        s /= 2.0
    hi = (bc * s).astype(fp8)
    lo = ((bc * s) - hi.astype(np.float64)).astype(fp8)
    bias2 = np.ascontiguousarray(np.stack([hi, lo]).reshape(1, 2, K))
    onesq = np.full((1, 2, 128), 1.0 / s, dtype=fp8)
    cq = centroids.astype(fp8)
    in_maps = []
    for c in range(N_CORES):
        sl = np.ascontiguousarray(xt[:, c * NC:(c + 1) * NC]).astype(fp8)
        in_maps.append({"xt": sl, "cent": cq, "bias2": bias2,
                        "onesq": onesq})
    return in_maps, bias_row


def _merge_host(outv, outi):
    """[KCM, 128, NT*8] device layout -> [NC, KCM, 8] candidates."""
    vals = outv.reshape(KCM, 128, NT, 8).transpose(2, 1, 0, 3).reshape(
        NC, KCM, 8)
    idxs = outi.reshape(KCM, 128, NT, 8).transpose(2, 1, 0, 3).reshape(
        NC, KCM, 8)
    return vals, idxs


def kernel(x: np.ndarray, centroids: np.ndarray) -> np.ndarray:
    from concourse.bass_utils import run_bass_kernel_spmd

    x = np.asarray(x, dtype=np.float32)
    centroids = np.asarray(centroids, dtype=np.float32)
    nc = _get_nc()

    in_maps, bias_row = _make_in_maps(x, centroids)
    res = run_bass_kernel_spmd(nc, in_maps, core_ids=list(range(N_CORES)))

    out = np.empty(N_FULL, dtype=np.int32)
    for c in range(N_CORES):
        vals, idxs = _merge_host(res.results[c]["outv"],
                                 res.results[c]["outi"])
        gi = _refine(x[c * NC:(c + 1) * NC], centroids, bias_row, vals, idxs)
        out[c * NC:(c + 1) * NC] = gi
    return out


def _refine(xs, centroids, bias_row, vals, idxs, top=16):
    """Re-score each row's top candidates exactly to undo fp8 noise."""
    n = xs.shape[0]
    fv = vals.reshape(n, KCM * 8)
    fi = (idxs.astype(np.int64)
          + (np.arange(KCM) * PAIR)[None, :, None]).reshape(n, KCM * 8)
    part = np.argpartition(-fv, top - 1, axis=1)[:, :top]
    cand = np.take_along_axis(fi, part, axis=1)          # [n, top] global idx
    out = np.empty(n, dtype=np.int32)
    bs = 4096
    for s in range(0, n, bs):
        e = min(s + bs, n)
        cb = cand[s:e]                                   # [b, top]
        cc = centroids.T[cb]                             # [b, top, D]
        sc = np.einsum("bd,btd->bt", xs[s:e], cc, dtype=np.float64)
        sc = sc + bias_row[cb]
        # argmax with ties -> smallest global index (first occurrence in k)
        best = sc.max(axis=1, keepdims=True)
        big = np.where(sc >= best, cb, np.iinfo(np.int64).max)
        out[s:e] = big.min(axis=1).astype(np.int32)
    return out
